# revision 4
# baseline (speedup 1.0000x reference)
"""FPN ROIAlign pooler (nn_Pooler) on 8 trn2 cores — TensorE version.

Strategy: data-parallel over RoIs. Host builds a channels-last bf16 pixel
table and per-box separable bilinear weights collapsed into dense per-chunk
weight matrices W[128px, 49bins]. Device: per box, load its footprint patch
via dynamic-offset strided DMAs into a pixel-on-partition tile
[128, nch, 256], then nch matmuls W^T @ patch accumulate pooled[49, 256] in
PSUM; evacuate (cast bf16) and DMA out. Boxes are sorted per level and dealt
round-robin so all 8 cores run one shared program; per-rank shape covers are
maxed over cores.
"""
import numpy as np
import ml_dtypes
from contextlib import ExitStack

from concourse import bacc, bass, mybir, tile, bass_utils

nbf16 = ml_dtypes.bfloat16
BF16 = mybir.dt.bfloat16
C = 256
OUT = 7
NBIN = OUT * OUT
N_CORES = 8
LVL_HW = [(200, 304), (100, 152), (50, 76), (25, 38)]
SCALES = (0.25, 0.125, 0.0625, 0.03125)
SEG_BASE = np.zeros((4, 2), np.int64)
_off = 0
for _l in range(4):
    for _b in range(2):
        SEG_BASE[_l, _b] = _off
        _off += LVL_HW[_l][0] * LVL_HW[_l][1]
TOTAL_PX = int(_off)
PAD_PX = 16384
TABLE_PX = TOTAL_PX + PAD_PX

WCLS = [8, 12, 16, 20, 24, 28, 32, 40, 48, 64, 96, 128]
WBLK_CHUNKS = 64       # W streaming block size (chunks)
OUT_GRP = 8            # slots per output staging group

_nc_cache = {}


def _geometry(boxes, bidx):
    """Per-box level routing + footprint + separable bilinear weights."""
    boxes32 = np.asarray(boxes, np.float32)
    b = np.asarray(bidx).astype(np.int64)
    N = boxes32.shape[0]

    x1, y1, x2, y2 = (boxes32[:, k] for k in range(4))
    area = (x2 - x1 + np.float32(1.0)) * (y2 - y1 + np.float32(1.0))
    s = np.sqrt(area)
    lv = np.floor(np.float32(4.0) + np.log2(s / np.float32(224.0)
                                            + np.float32(1e-6)))
    lvl = (np.clip(lv, 2.0, 5.0)).astype(np.int64) - 2

    scs = np.array(SCALES)[lvl]
    Wl = np.array([hw[1] for hw in LVL_HW])[lvl]
    Hl = np.array([hw[0] for hw in LVL_HW])[lvl]
    x1s = boxes32[:, 0].astype(np.float64) * scs
    y1s = boxes32[:, 1].astype(np.float64) * scs
    x2s = boxes32[:, 2].astype(np.float64) * scs
    y2s = boxes32[:, 3].astype(np.float64) * scs
    bin_w = np.maximum(x2s - x1s, 1.0) / OUT
    bin_h = np.maximum(y2s - y1s, 1.0) / OUT
    grid = (np.arange(OUT)[:, None]
            + np.array([0.25, 0.75])[None, :]).reshape(-1)
    xs = x1s[:, None] + bin_w[:, None] * grid[None, :]     # [N,14]
    ys = y1s[:, None] + bin_h[:, None] * grid[None, :]
    vx = (xs >= -1.0) & (xs <= Wl[:, None])
    vy = (ys >= -1.0) & (ys <= Hl[:, None])
    xc = np.clip(xs, 0.0, (Wl - 1)[:, None])
    yc = np.clip(ys, 0.0, (Hl - 1)[:, None])
    x0c = np.minimum(np.floor(xc).astype(np.int64), (Wl - 2)[:, None])
    y0c = np.minimum(np.floor(yc).astype(np.int64), (Hl - 2)[:, None])
    lx = xc - x0c
    ly = yc - y0c

    xmin = x0c.min(1)
    ymin = y0c.min(1)
    w_ext = x0c.max(1) + 1 - xmin + 1
    h_ext = y0c.max(1) + 1 - ymin + 1

    # separable per-pixel weights: A [N,7,h_ext], B [N,7,w_ext] (ragged)
    A_list, B_list = [], []
    for i in range(N):
        A = np.zeros((OUT, int(h_ext[i])), np.float64)
        Bm = np.zeros((OUT, int(w_ext[i])), np.float64)
        ry = y0c[i] - ymin[i]
        rx = x0c[i] - xmin[i]
        wy0 = 0.5 * vy[i] * (1.0 - ly[i])
        wy1 = 0.5 * vy[i] * ly[i]
        wx0 = 0.5 * vx[i] * (1.0 - lx[i])
        wx1 = 0.5 * vx[i] * lx[i]
        for sy in range(2 * OUT):
            by = sy // 2
            A[by, ry[sy]] += wy0[sy]
            A[by, ry[sy] + 1] += wy1[sy]
        for sx in range(2 * OUT):
            bx = sx // 2
            Bm[bx, rx[sx]] += wx0[sx]
            Bm[bx, rx[sx] + 1] += wx1[sx]
        A_list.append(A.astype(np.float32))
        B_list.append(Bm.astype(np.float32))

    return dict(lvl=lvl, b=b, xmin=xmin, ymin=ymin, w_ext=w_ext,
                h_ext=h_ext, A=A_list, B=B_list)


def _plan(geo):
    """Sort + deal boxes, compute per-rank shape covers."""
    lvl = geo["lvl"]
    w_ext = geo["w_ext"]
    h_ext = geo["h_ext"]
    wclass = np.array([min(c for c in WCLS if c >= w) for w in w_ext])

    slots = []          # list of dict(lvl, wc, k, nch, Wl)
    slot_boxes = []     # list of [box_id or -1 per core]
    for l in range(4):
        ids = np.nonzero(lvl == l)[0]
        order = ids[np.lexsort((-h_ext[ids], -wclass[ids]))]
        n_ranks = -(-len(order) // N_CORES)
        for j in range(n_ranks):
            grp = order[j * N_CORES:(j + 1) * N_CORES]
            cores_boxes = [-1] * N_CORES
            for c, bid in enumerate(grp):
                cores_boxes[c] = int(bid)
            wc = int(wclass[grp].max())
            k = 128 // wc
            nch = int(max(-(-int(h_ext[g]) // k) for g in grp))
            slots.append(dict(lvl=l, wc=wc, k=k, nch=nch,
                              Wl=LVL_HW[l][1]))
            slot_boxes.append(cores_boxes)
    return slots, slot_boxes


def _host_prep(f0, f1, f2, f3, boxes, bidx):
    geo = _geometry(boxes, bidx)
    slots, slot_boxes = _plan(geo)

    # channels-last bf16 table
    segs = []
    for f in (f0, f1, f2, f3):
        fa = np.asarray(f, np.float32)
        for bb in range(2):
            segs.append(np.transpose(fa[bb], (1, 2, 0)).reshape(-1, C))
    segs.append(np.zeros((PAD_PX, C), np.float32))
    table = np.ascontiguousarray(
        np.concatenate(segs, 0)).astype(nbf16).reshape(-1)

    tot_chunks = sum(s["nch"] for s in slots)
    n_slots = len(slots)

    metas = [np.zeros((1, tot_chunks), np.int32) for _ in range(N_CORES)]
    whosts = [np.zeros((128, tot_chunks * NBIN), np.float32)
              for _ in range(N_CORES)]

    gbase = 0
    for si, s in enumerate(slots):
        k, wc, nch, Wl = s["k"], s["wc"], s["nch"], s["Wl"]
        kwc = k * wc
        for core in range(N_CORES):
            bid = slot_boxes[si][core]
            if bid < 0:
                continue
            seg = SEG_BASE[geo["lvl"][bid], geo["b"][bid]]
            base_px = seg + geo["ymin"][bid] * Wl + geo["xmin"][bid]
            for cch in range(nch):
                metas[core][0, gbase + cch] = (base_px + cch * k * Wl) * C
            # weights: rows = nch*k (pad past h_ext), cols = wc (pad past w)
            h = int(geo["h_ext"][bid])
            w = int(geo["w_ext"][bid])
            A = geo["A"][bid]          # [7, h]
            B = geo["B"][bid]          # [7, w]
            Ap = np.zeros((OUT, nch * k), np.float32)
            Ap[:, :h] = A
            Bp = np.zeros((OUT, wc), np.float32)
            Bp[:, :w] = B
            # T[r, x, by, bx] = Ap[by,r]*Bp[bx,x]
            T = np.einsum('ar,bx->rxab', Ap, Bp).reshape(nch, kwc, NBIN)
            wh = whosts[core]
            for cch in range(nch):
                wh[:kwc, (gbase + cch) * NBIN:(gbase + cch + 1) * NBIN] = \
                    T[cch]
        gbase += nch
    assert gbase == tot_chunks

    whosts = [w.astype(nbf16) for w in whosts]
    key = tuple((s["lvl"], s["wc"], s["nch"]) for s in slots)
    return table, metas, whosts, slots, slot_boxes, tot_chunks, n_slots, key


def _build_nc(slots):
    tot_chunks = sum(s["nch"] for s in slots)
    n_slots = len(slots)
    nc = bacc.Bacc("TRN2", target_bir_lowering=False, debug=False,
                   num_devices=N_CORES)
    table_d = nc.dram_tensor("table", [TABLE_PX * C], BF16,
                             kind="ExternalInput")
    meta_d = nc.dram_tensor("meta", [1, tot_chunks], mybir.dt.int32,
                            kind="ExternalInput")
    w_d = nc.dram_tensor("wts", [128, tot_chunks * NBIN], BF16,
                         kind="ExternalInput")
    out_d = nc.dram_tensor("out", [NBIN, n_slots, C], BF16,
                           kind="ExternalOutput")

    with tile.TileContext(nc) as tc, ExitStack() as ctx:
        sbm = ctx.enter_context(tc.tile_pool(name="sbm", bufs=1))
        sbw = ctx.enter_context(tc.tile_pool(name="sbw", bufs=2))
        sbp = ctx.enter_context(tc.tile_pool(name="sbp", bufs=4))
        sbo = ctx.enter_context(tc.tile_pool(name="sbo", bufs=2))
        psp = ctx.enter_context(tc.psum_pool(name="psp", bufs=8))

        meta_t = sbm.tile([1, tot_chunks], mybir.dt.int32)
        nc.sync.dma_start(out=meta_t[:], in_=meta_d.ap())

        # W block schedule: block i covers chunks [i*WBLK, ...)
        n_blk = -(-tot_chunks // WBLK_CHUNKS)
        wtiles = [None] * n_blk

        def get_wblk(blk):
            if wtiles[blk] is None:
                lo = blk * WBLK_CHUNKS
                hi = min(tot_chunks, lo + WBLK_CHUNKS)
                t = sbw.tile([128, (hi - lo) * NBIN], BF16)
                nc.sync.dma_start(
                    out=t[:], in_=w_d.ap()[:, lo * NBIN:hi * NBIN])
                wtiles[blk] = (t, lo)
            return wtiles[blk]

        stage = None
        gc = 0
        for si, s in enumerate(slots):
            k, wc, nch, Wl = s["k"], s["wc"], s["nch"], s["Wl"]
            kwc = k * wc
            eng = nc.sync if (si % 2 == 0) else nc.scalar
            patch = sbp.tile([128, nch, C], BF16)
            if kwc < 128:
                mstart = (kwc // 32) * 32
                nc.vector.memset(patch[mstart:128, :, :], 0.0)
            for cch in range(nch):
                off = eng.value_load(meta_t[0:1, gc + cch:gc + cch + 1])
                src = bass.AP(tensor=table_d, offset=off,
                              ap=[[Wl * C, k], [C, wc], [1, C]])
                eng.dma_start(out=patch[:kwc, cch, :], in_=src)
            ps = psp.tile([NBIN, C], mybir.dt.float32)
            for cch in range(nch):
                blk, lo = divmod(gc + cch, WBLK_CHUNKS)
                wt, wlo = get_wblk(blk)
                g_local = gc + cch - wlo
                nc.tensor.matmul(
                    out=ps[:],
                    lhsT=wt[:, g_local * NBIN:(g_local + 1) * NBIN],
                    rhs=patch[:, cch, :],
                    start=(cch == 0), stop=(cch == nch - 1))
            gi = si % OUT_GRP
            if gi == 0:
                gsz = min(OUT_GRP, n_slots - si)
                stage = sbo.tile([NBIN, gsz, C], BF16)
            nc.vector.tensor_scalar_mul(stage[:, gi, :], ps[:], 1.0)
            if gi == gsz - 1:
                g0 = si - gi
                nc.scalar.dma_start(
                    out=out_d.ap()[:, g0:g0 + gsz, :], in_=stage[:])
            gc += nch
        # free W tiles ref (tiles auto-managed by pool)
    nc.compile()
    return nc


LAST_RESULT = None


def kernel(f0, f1, f2, f3, boxes, box_batch_idx):
    global LAST_RESULT
    (table, metas, whosts, slots, slot_boxes, tot_chunks, n_slots,
     key) = _host_prep(f0, f1, f2, f3, boxes, box_batch_idx)
    if key not in _nc_cache:
        _nc_cache[key] = _build_nc(slots)
    nc = _nc_cache[key]
    in_maps = [{"table": table, "meta": metas[i], "wts": whosts[i]}
               for i in range(N_CORES)]
    res = bass_utils.run_bass_kernel_spmd(nc, in_maps,
                                          core_ids=list(range(N_CORES)))
    LAST_RESULT = res

    outfull = np.zeros((1024, NBIN, C), np.float32)
    for core in range(N_CORES):
        r = np.asarray(res.results[core]["out"]).astype(np.float32)
        for si in range(n_slots):
            bid = slot_boxes[si][core]
            if bid >= 0:
                outfull[bid] = r[:, si, :]
    return np.ascontiguousarray(
        outfull.transpose(0, 2, 1).reshape(1024, C, OUT, OUT))
